# revision 3
# baseline (speedup 1.0000x reference)
"""Cosine-similarity attention (B=2, H=16, S=2048, D=64) on 8 TRN2 NeuronCores.

Sharding: batch*head (32 heads) split 4-heads-per-core across 8 cores; each core
computes full attention for its heads. No cross-core communication.

Per-core pipeline (all matmul operands fp16; PSUM fp32 for matmuls):
  prep:  load q/k natural [128, 16*64]; row norms via DVE square+reduce,
         ACT sqrt + DVE reciprocal + one Newton step; scale+cast to fp16;
         PE-transpose to QnT/KnT [64, 2048] fp16.  V -> Vp [128, 16*65] fp16
         with a ones column (gives softmax denominators via the PV matmul).
  main (per head h, q-block j of 512):
    - ST scores: psum[k=128, q=1024-pair] = KnT_kt^T @ QnT_j  (fp16, 1cyc/row)
    - ACT exp(10*s) psum->sbuf fp16 (temperature folded into ACT scale)
    - PV' with ones column -> psum [65, 512]; row 64 = softmax denominators
    - PE-transpose PV' -> [128, 65]; DVE reciprocal -> inv[q]; scale -> out
    - PE-transpose exp tiles -> natural [q, k] psum fp16; evacuate with fused
      normalize (x * inv[q]) split across DVE/ACT -> fp16 probs -> DMA
  score is written to DRAM as fp16 (values are fp16-rounded anyway); host
  upcasts to fp32.
"""
import sys

sys.path.insert(0, '/opt/trn_rl_repo')

from contextlib import ExitStack

import numpy as np

import concourse.bass as bass  # noqa: F401  (engine types via nc)
import concourse.tile as tile
from concourse import bacc, mybir
from concourse.bass_utils import run_bass_kernel_spmd
from concourse.masks import make_identity

F32 = mybir.dt.float32
F16 = mybir.dt.float16
AF = mybir.ActivationFunctionType

B, H, S, D = 2, 16, 2048, 64
N_CORES = 8
HPC = (B * H) // N_CORES          # heads per core = 4
T = S // 128                      # 16 q/k tiles of 128
JB = 512                          # q-block width
NJ = S // JB                      # 4 q-blocks
KTP = T // 2                      # 8 kt-pairs
TEMP_INV = 10.0                   # 1/temperature
ACT_EVAC_EVERY = 5                # every Nth normalize-evac goes to ScalarE


def build_program():
    nc = bacc.Bacc("TRN2", target_bir_lowering=False, debug=False,
                   num_devices=N_CORES)
    q_d = nc.dram_tensor("q", [HPC, S, D], F32, kind="ExternalInput").ap()
    k_d = nc.dram_tensor("k", [HPC, S, D], F32, kind="ExternalInput").ap()
    v_d = nc.dram_tensor("v", [HPC, S, D], F32, kind="ExternalInput").ap()
    out_d = nc.dram_tensor("out", [HPC, S, D], F32, kind="ExternalOutput").ap()
    score_d = nc.dram_tensor("score", [HPC, S, S], F16, kind="ExternalOutput").ap()

    with tile.TileContext(nc) as tc, ExitStack() as ctx:
        const = ctx.enter_context(tc.tile_pool(name="const", bufs=1))
        io = ctx.enter_context(tc.tile_pool(name="io", bufs=HPC))
        prep = ctx.enter_context(tc.tile_pool(name="prep", bufs=2))
        opnd = ctx.enter_context(tc.tile_pool(name="opnd", bufs=2))
        expp = ctx.enter_context(tc.tile_pool(name="expp", bufs=T + 4))
        pnat = ctx.enter_context(tc.tile_pool(name="pnat", bufs=4))
        outp = ctx.enter_context(tc.tile_pool(name="outp", bufs=2))
        invp = ctx.enter_context(tc.tile_pool(name="invp", bufs=10))
        ps_qk = ctx.enter_context(tc.tile_pool(name="ps_qk", bufs=2, space="PSUM"))
        ps_tr = ctx.enter_context(tc.tile_pool(name="ps_tr", bufs=2, space="PSUM"))
        ps_pv = ctx.enter_context(tc.tile_pool(name="ps_pv", bufs=1, space="PSUM"))
        ps_ot = ctx.enter_context(tc.tile_pool(name="ps_ot", bufs=1, space="PSUM"))

        id16 = const.tile([128, 128], F16, tag="id16")
        id32 = const.tile([128, 128], F32, tag="id32")
        make_identity(nc, id16[:])
        make_identity(nc, id32[:])

        # ---------- phase A: load q/k/v, compute all row norms ----------
        q_nat, k_nat, vp = [], [], []
        s2 = const.tile([128, 2 * HPC * T], F32, tag="s2")  # [128, 128]
        for h in range(HPC):
            qn = io.tile([128, T * D], F32, tag="qnat")
            kn = io.tile([128, T * D], F32, tag="knat")
            nc.sync.dma_start(qn[:].rearrange("p (t d) -> p t d", d=D),
                              q_d[h].rearrange("(t p) d -> p t d", p=128))
            nc.sync.dma_start(kn[:].rearrange("p (t d) -> p t d", d=D),
                              k_d[h].rearrange("(t p) d -> p t d", p=128))
            q_nat.append(qn)
            k_nat.append(kn)

            vpt = io.tile([128, T * (D + 1)], F16, tag="vp")
            vp3 = vpt[:].rearrange("p (t d) -> p t d", d=D + 1)
            nc.gpsimd.dma_start(vp3[:, :, 0:D],
                                v_d[h].rearrange("(t p) d -> p t d", p=128))
            nc.gpsimd.memset(vp3[:, :, D:D + 1], 1.0)
            vp.append(vpt)

            for ti, xn in ((2 * h, qn), (2 * h + 1, kn)):
                sq = prep.tile([128, T * D], F32, tag="sq")
                nc.vector.tensor_mul(sq[:], xn[:], xn[:])
                nc.vector.tensor_reduce(
                    s2[:, ti * T:(ti + 1) * T],
                    sq[:].rearrange("p (t d) -> p t d", d=D),
                    axis=mybir.AxisListType.X, op=mybir.AluOpType.add)

        # inv = rsqrt(s2), via ACT sqrt seed + reciprocal + 1 Newton step
        nrm = const.tile([128, 2 * HPC * T], F32, tag="nrm")
        nc.scalar.activation(nrm[:], s2[:], AF.Sqrt, bias=0.0, scale=1.0)
        inv0 = const.tile([128, 2 * HPC * T], F32, tag="inv0")
        nc.vector.reciprocal(inv0[:], nrm[:])
        t1 = const.tile([128, 2 * HPC * T], F32, tag="t1")
        t2 = const.tile([128, 2 * HPC * T], F32, tag="t2")
        t3 = const.tile([128, 2 * HPC * T], F32, tag="t3")
        nc.vector.tensor_mul(t1[:], inv0[:], inv0[:])        # z^2
        nc.vector.tensor_mul(t2[:], t1[:], s2[:])            # x z^2
        nc.vector.tensor_scalar(t3[:], t2[:], -0.5, 1.5,
                                mybir.AluOpType.mult, mybir.AluOpType.add)
        inv = const.tile([128, 2 * HPC * T], F32, tag="inv")
        nc.vector.tensor_mul(inv[:], inv0[:], t3[:])         # z(1.5-0.5xz^2)

        # ---------- per-head prep + main loop ----------
        qnT, knT = {}, {}

        def prep_head(h):
            for ti, xn, dstmap, tg in ((2 * h, q_nat[h], qnT, "qnT"),
                                       (2 * h + 1, k_nat[h], knT, "knT")):
                x16 = prep.tile([128, T * D], F16, tag="x16")
                for t in range(T):
                    nc.vector.tensor_scalar_mul(
                        x16[:, t * D:(t + 1) * D], xn[:, t * D:(t + 1) * D],
                        inv[:, ti * T + t:ti * T + t + 1])
                xT = opnd.tile([128, S], F16, tag=tg)
                for g in range(2):  # 8 transposes per psum bank-tile
                    tr = ps_tr.tile([128, 1024], F16, tag="tr")
                    for kk in range(8):
                        t = g * 8 + kk
                        nc.tensor.transpose(tr[0:64, kk * 128:(kk + 1) * 128],
                                            x16[:, t * D:(t + 1) * D], id16[:])
                    nc.vector.tensor_copy(xT[0:64, g * 1024:(g + 1) * 1024],
                                          tr[0:64, :])
                # duplicate to partitions 64-127 for 2x row-tiled matmuls
                nc.sync.dma_start(xT[64:128, :], xT[0:64, :])
                dstmap[h] = xT

        evac_ctr = [0]
        pending = []  # deferred probs-transpose stages (one iteration behind)

        def probs_stage(h, j, exps, invs):
            for qt in range(4):
                pn = pnat.tile([128, S], F16, tag="pn")
                for g in range(2):
                    tr = ps_tr.tile([128, 1024], F16, tag="tr")
                    for kk in range(8):
                        kt = g * 8 + kk
                        src = exps[kt // 2][:, (kt % 2) * JB + qt * 128:
                                            (kt % 2) * JB + (qt + 1) * 128]
                        nc.tensor.transpose(tr[:, kk * 128:(kk + 1) * 128],
                                            src, id16[:])
                    dst = pn[:, g * 1024:(g + 1) * 1024]
                    evac_ctr[0] += 1
                    if evac_ctr[0] % ACT_EVAC_EVERY == 0:
                        nc.scalar.mul(dst, tr[:], invs[qt][:])
                    else:
                        nc.vector.tensor_scalar_mul(dst, tr[:], invs[qt][:])
                q0 = j * JB + qt * 128
                nc.sync.dma_start(score_d[h, q0:q0 + 128, :], pn[:])

        prep_head(0)
        for h in range(HPC):
            if h + 1 < HPC:
                prep_head(h + 1)
            out_sb = outp.tile([128, T * D], F32, tag="outsb")
            for j in range(NJ):
                # --- deferred probs stage from previous iteration (releases
                # the previous iteration's exp tiles before new ones alloc) ---
                if pending:
                    probs_stage(*pending.pop())
                # --- ST scores + exp ---
                exps = []
                for p in range(KTP):
                    qk = ps_qk.tile([128, 1024], F32, tag="qk")
                    for half in range(2):
                        kt = 2 * p + half
                        base = 64 * half
                        nc.tensor.matmul(
                            qk[:, half * JB:(half + 1) * JB],
                            lhsT=knT[h][base:base + 64, kt * 128:(kt + 1) * 128],
                            rhs=qnT[h][base:base + 64, j * JB:(j + 1) * JB],
                            start=True, stop=True, tile_position=(base, 0))
                    e = expp.tile([128, 1024], F16, tag="expst")
                    nc.scalar.activation(e[:], qk[:], AF.Exp,
                                         bias=0.0, scale=TEMP_INV)
                    exps.append(e)
                # --- PV' (ones column -> denominators in row 64) ---
                pv = ps_pv.tile([65, JB], F32, tag="pv")
                for kt in range(T):
                    nc.tensor.matmul(
                        pv[:],
                        lhsT=vp[h][:].rearrange("p (t d) -> p t d", d=D + 1)[:, kt, :],
                        rhs=exps[kt // 2][:, (kt % 2) * JB:(kt % 2 + 1) * JB],
                        start=(kt == 0), stop=(kt == T - 1))
                pv_sb = outp.tile([65, JB], F32, tag="pvsb")
                nc.vector.tensor_copy(pv_sb[:], pv[:])
                # --- out path: transpose [65,512] -> 4x [128,65], scale ---
                ot = ps_ot.tile([128, 4 * 65], F32, tag="ot")
                invs = []
                for qt in range(4):
                    nc.tensor.transpose(ot[:, qt * 65:(qt + 1) * 65],
                                        pv_sb[:, qt * 128:(qt + 1) * 128],
                                        id32[0:65, 0:65])
                for qt in range(4):
                    iv = invp.tile([128, 1], F32, tag="inv_q")
                    nc.vector.reciprocal(iv[:], ot[:, qt * 65 + 64:qt * 65 + 65])
                    invs.append(iv)
                    tg = j * 4 + qt
                    nc.vector.tensor_scalar_mul(
                        out_sb[:, tg * D:(tg + 1) * D],
                        ot[:, qt * 65:qt * 65 + 64], iv[:])
                pending.append((h, j, exps, invs))
            nc.sync.dma_start(out_d[h].rearrange("(t p) d -> p t d", p=128),
                              out_sb[:].rearrange("p (t d) -> p t d", d=D))
        while pending:
            probs_stage(*pending.pop())

    nc.compile()
    return nc


_NC = None


def _get_program():
    global _NC
    if _NC is None:
        _NC = build_program()
    return _NC


def run_sharded(q, k, v, trace=False):
    """q/k/v: [B,H,S,D] fp32. Returns (out, score, exec_time_ns)."""
    nc = _get_program()
    q_s = np.ascontiguousarray(q.reshape(N_CORES, HPC, S, D), dtype=np.float32)
    k_s = np.ascontiguousarray(k.reshape(N_CORES, HPC, S, D), dtype=np.float32)
    v_s = np.ascontiguousarray(v.reshape(N_CORES, HPC, S, D), dtype=np.float32)
    in_maps = [{"q": q_s[i], "k": k_s[i], "v": v_s[i]} for i in range(N_CORES)]
    res = run_bass_kernel_spmd(nc, in_maps, list(range(N_CORES)), trace=trace)
    out = np.stack([res.results[i]["out"] for i in range(N_CORES)])
    score = np.stack([res.results[i]["score"] for i in range(N_CORES)])
    out = out.reshape(B, H, S, D).astype(np.float32)
    score = score.reshape(B, H, S, S).astype(np.float32)
    return out, score, res.exec_time_ns


def kernel(q, k, v):
    out, score, _ = run_sharded(np.asarray(q), np.asarray(k), np.asarray(v))
    return out, score
